# revision 14
# baseline (speedup 1.0000x reference)
"""Trainium2 Bass kernel for spatial-reduction attention (nn_Attention_11269994184820).

Strategy: head-parallel over 8 cores (8 heads). Each core computes one head's
attention for all 4 batches in a transposed layout (k-tokens on partitions),
then a per-batch AllToAll redistributes head-outputs to token-slices so the
output projection overlaps later batches' compute.

Key techniques:
  - exp(qk + rel) = exp(qk) * exp(rel): exp(rel) precomputed on host in fp16,
    ACT does Exp straight from PSUM (fused evacuation), DVE does an all-SBUF
    contiguous fp16 multiply (2x/4x perf mode) instead of a fp32 PSUM add.
  - Depthwise conv + BN folded to per-channel tap-sums on the Pool engine;
    k-bias dropped (softmax-invariant), v-bias folded into proj bias (exact).
  - fp16 x / weights end-to-end.
  - PE kept continuously busy (p-state!): software-pipelined qc schedule
    [QK0 QK1 QK2 AV0 QK3 AV1 AV2 AV3], next batch's q/k/v projections and
    previous batch's output projection interleaved into the attention loop.
  - Per-batch AllToAll chunks overlap collective + proj with compute.

Per-core layouts (core = head h):
  xt[b]     [2, 128, 4096] f16   x^T channel-chunked
  erp[g]    [128, 8192] f16      [p, qc*1024+u*512+q] = er[(2g+u)*128+p, qc*512+q]
  qrep      [128, 4096] f16      qT replicated 4x on partitions (row-packed QK)
  kstrip    [128, 128] f16 x2    kT chunks at partition strips
  scores    PSUM [128 k, 2x512 q] per kc-pair; ACT Exp -> et f16; DVE *= erp
  AV        [v|1] stationary accumulates [33, 512] over 8 kc; row 32 = denom
  outu[b]   [8, 33, 512] -> AllToAll -> recv[b]; normalize + proj 512 rows
"""

import sys

if "/opt/trn_rl_repo" not in sys.path:
    sys.path.insert(0, "/opt/trn_rl_repo")

from contextlib import ExitStack

import numpy as np
from ml_dtypes import bfloat16

import concourse.bacc as bacc
import concourse.bass as bass
import concourse.mybir as mybir
import concourse.tile as tile
from concourse.bass_utils import run_bass_kernel_spmd

F32 = mybir.dt.float32
F16 = mybir.dt.float16
BF16 = mybir.dt.bfloat16
N_CORES = 8
B, N, C = 4, 4096, 256
HEADS, DH, SR, NK = 8, 32, 2, 1024
BN_EPS = 1e-5

_CACHE = {}


def _build_nc():
    nc = bacc.Bacc("TRN2", target_bir_lowering=False, debug=False, num_devices=N_CORES)

    def din(name, shape, dt=F32):
        return nc.dram_tensor(name, list(shape), dt, kind="ExternalInput").ap()

    xt_d = din("xt", [B, 2, 128, N], F16)
    erp_d = din("erp", [4, 128, 2 * N], BF16)
    qw_d = din("qw", [2, 128, 128], F16)
    kw_d = din("kw", [2, 128, 32], F16)
    vw_d = din("vw", [2, 128, 32], F16)
    tapw_d = din("tapw", [2, 128, 4])
    pwt_d = din("pwt", [2, 128, 256], F16)
    pb_d = din("pbrep", [128, 256])
    out_d = nc.dram_tensor("out", [B, 512, 256], F32, kind="ExternalOutput").ap()

    AF = mybir.ActivationFunctionType
    OP = mybir.AluOpType

    with tile.TileContext(nc) as tc, ExitStack() as ctx:
        pool = ctx.enter_context(tc.tile_pool(name="main", bufs=1))
        p_dram = ctx.enter_context(tc.tile_pool(name="dram", bufs=1, space="DRAM"))
        ps_sc = ctx.enter_context(tc.tile_pool(name="ps_sc", bufs=2, space="PSUM"))
        ps_av = ctx.enter_context(tc.tile_pool(name="ps_av", bufs=2, space="PSUM"))
        ps_mi = ctx.enter_context(tc.tile_pool(name="ps_mi", bufs=2, space="PSUM"))

        def const_tile(src, shape, tag, dt=F32):
            t = pool.tile(shape, dt, tag=tag)
            nc.sync.dma_start(t[:], src)
            return t

        # ---------------- per-batch input loads (prefetch-friendly) --------
        xt_sb = {}

        def load_xt(b):
            ts = []
            for cc in range(2):
                t = pool.tile([128, N], F16, tag=f"xt{cc}", bufs=2,
                              name=f"xt{b}{cc}")
                nc.sync.dma_start(t[:], xt_d[b, cc])
                ts.append(t)
            xt_sb[b] = ts

        load_xt(0)
        qw_sb = [const_tile(qw_d[cc], [128, 128], f"qw{cc}", F16) for cc in range(2)]
        kw_sb = [const_tile(kw_d[cc], [128, 32], f"kw{cc}", F16) for cc in range(2)]
        vw_sb = [const_tile(vw_d[cc], [128, 32], f"vw{cc}", F16) for cc in range(2)]
        tap_sb = [const_tile(tapw_d[cc], [128, 4], f"tap{cc}") for cc in range(2)]
        pwt_sb = [const_tile(pwt_d[cc], [128, 256], f"pwt{cc}", F16)
                  for cc in range(2)]
        pb_sb = const_tile(pb_d[:], [128, 256], "pbrep")
        erp_sb = [const_tile(erp_d[g], [128, 2 * N], f"erp{g}", BF16)
                  for g in range(4)]

        outu_d = p_dram.tile([B, 8, 33, 512], F32, tag="outu")
        recv_d = p_dram.tile([B, 8, 33, 512], F32, tag="recv")
        den_d = p_dram.tile([B, 128, 32], F32, tag="den_d")
        recip_d = p_dram.tile([B, 128, 32], F32, tag="recip_d")

        def prep_chunks(b):
            """Emission chunks for batch b's projections (interleaved into
            the previous batch's attention loop to keep PE streaming)."""
            xts = xt_sb.pop(b)
            xk = [pool.tile([128, NK], F16, tag=f"xk{cc}", bufs=2,
                            name=f"xk{b}{cc}") for cc in range(2)]
            qrep = pool.tile([128, N], F16, tag="qrep", bufs=2, name=f"qrep{b}")
            kstrip = [pool.tile([128, 128], F16, tag=f"ks{grp}", bufs=2,
                                name=f"ks{b}{grp}") for grp in range(2)]
            vsb = [pool.tile([128, 33], BF16, tag=f"v{kc}", bufs=2,
                             name=f"vt{b}{kc}") for kc in range(8)]

            def c_xk_q():
                # depthwise 2x2/2 conv + BN as per-channel tap-sums (Pool)
                for cc in range(2):
                    view = xts[cc][:].rearrange(
                        "p (i a j b2) -> p a b2 i j", i=32, a=2, j=32, b2=2)
                    tv = xk[cc][:].rearrange("p (i j) -> p i j", i=32)
                    for tap in range(4):
                        di, dj = tap // 2, tap % 2
                        src = view[:, di, dj]
                        sc = tap_sb[cc][:, tap:tap + 1]
                        if tap == 0:
                            nc.vector.tensor_scalar(tv, src, sc, None,
                                                    op0=OP.mult)
                        else:
                            nc.vector.scalar_tensor_tensor(
                                tv, src, sc, tv, op0=OP.mult, op1=OP.add)
                # q projection -> qT replicated 4x along partitions, f16
                for ncc in range(8):
                    psq = ps_mi.tile([128, 512], F32, tag="mi",
                                     name=f"q{b}{ncc}")
                    for cc in range(2):
                        nc.tensor.matmul(psq[:], qw_sb[cc][:],
                                         xts[cc][:, ncc * 512:(ncc + 1) * 512],
                                         start=(cc == 0), stop=(cc == 1))
                    nc.vector.tensor_copy(qrep[:, ncc * 512:(ncc + 1) * 512],
                                          psq[:])

            def c_k():
                for grp in range(2):
                    psk = ps_mi.tile([128, 128], F32, tag="mi",
                                     name=f"k{b}{grp}")
                    for s in range(4):
                        kc = grp * 4 + s
                        for cc in range(2):
                            nc.tensor.matmul(
                                psk[32 * s:32 * (s + 1), :], kw_sb[cc][:],
                                xk[cc][:, kc * 128:(kc + 1) * 128],
                                start=(cc == 0), stop=(cc == 1),
                                tile_position=(0, 32 * s))
                    nc.vector.tensor_copy(kstrip[grp][:], psk[:])

            def c_v(h):
                for kc in range(4 * h, 4 * h + 4):
                    psv = ps_mi.tile([128, 32], F32, tag="mi",
                                     name=f"v{b}{kc}")
                    for cc in range(2):
                        nc.tensor.matmul(
                            psv[:], xk[cc][:, kc * 128:(kc + 1) * 128],
                            vw_sb[cc][:], start=(cc == 0), stop=(cc == 1))
                    nc.vector.tensor_copy(vsb[kc][:, 0:32], psv[:])
                    nc.vector.memset(vsb[kc][:, 32:33], 1.0)

            return (qrep, kstrip, vsb), [c_xk_q, c_k,
                                         lambda: c_v(0), lambda: c_v(1)]

        def attention_batch(b, ctxb, hooks):
            qrep, kstrip, vsb = ctxb
            for qc in range(8):
                av = ps_av.tile([33, 512], F32, tag="av", name=f"av{b}{qc}")
                ets = [None] * 4

                def qk(g):
                    pssc = ps_sc.tile([128, 1024], F32, tag="sc",
                                      name=f"sc{b}{qc}{g}")
                    for u in range(2):
                        kc = 2 * g + u
                        s = kc % 4
                        nc.tensor.matmul(
                            pssc[:, u * 512:(u + 1) * 512],
                            kstrip[kc // 4][32 * s:32 * (s + 1), :],
                            qrep[32 * s:32 * (s + 1),
                                 qc * 512:(qc + 1) * 512],
                            start=True, stop=True, tile_position=(32 * s, 0))
                    et = pool.tile([128, 1024], BF16, tag="et", bufs=3,
                                   name=f"et{b}{qc}{g}")
                    nc.scalar.activation(et[:], pssc[:], AF.Exp)
                    nc.vector.tensor_tensor(
                        et[:], et[:],
                        erp_sb[g][:, qc * 1024:(qc + 1) * 1024], op=OP.mult)
                    ets[g] = et

                def avm(g):
                    for u in range(2):
                        kc = 2 * g + u
                        nc.tensor.matmul(av[:], vsb[kc][:],
                                         ets[g][:, u * 512:(u + 1) * 512],
                                         start=(kc == 0), stop=(kc == 7))

                # software-pipelined: AV(g) trails QK(g+2) so PE never waits
                qk(0); qk(1); qk(2); avm(0); qk(3); avm(1); avm(2); avm(3)

                ou = pool.tile([33, 512], F32, tag="ou", bufs=3,
                               name=f"ou{b}{qc}")
                nc.vector.tensor_copy(ou[:], av[:])
                nc.sync.dma_start(outu_d[b, qc], ou[:])

                for fn in hooks.get(qc, ()):
                    fn()

        def exchange_batch(b):
            nc.gpsimd.collective_compute(
                "AllToAll", mybir.AluOpType.bypass,
                replica_groups=[list(range(N_CORES))],
                ins=[outu_d[b].opt()], outs=[recv_d[b].opt()])

        def proj_pre(b):
            # assemble [4 heads x 32 dh, 512 tok] x2 + denominators
            lhs = [pool.tile([128, 512], F32, tag=f"lhs{i}", bufs=2,
                             name=f"lhs{b}{i}") for i in range(2)]
            for s in range(8):
                nc.gpsimd.dma_start(
                    lhs[s // 4][32 * (s % 4):32 * (s % 4 + 1), :],
                    recv_d[b, s, 0:32, :])
            # denominators reshaped to [128, 32] so reciprocal streams 32,
            # not 512, free elements (DVE cost is per free-elem)
            nc.gpsimd.dma_start(
                den_d[b].rearrange("(s i) j -> s i j", s=8),
                recv_d[b, :, 32, :].rearrange("s (i j) -> s i j", i=16))
            den = pool.tile([128, 32], F32, tag="den", bufs=2, name=f"den{b}")
            nc.gpsimd.dma_start(den[:], den_d[b])
            recip = pool.tile([128, 32], F32, tag="recip", bufs=2,
                              name=f"recip{b}")
            nc.vector.reciprocal(recip[:], den[:])
            nc.gpsimd.dma_start(recip_d[b], recip[:])
            bcr = [pool.tile([128, 512], F32, tag=f"bcr{i}", bufs=2,
                             name=f"bcr{b}{i}") for i in range(2)]
            for s in range(8):
                nc.gpsimd.dma_start(
                    bcr[s // 4][32 * (s % 4):32 * (s % 4 + 1), :],
                    recip_d[b, 16 * s:16 * (s + 1), :]
                    .rearrange("i j -> (i j)").partition_broadcast(32))
            lhsh = [pool.tile([128, 512], F16, tag=f"lhsh{i}", bufs=2,
                              name=f"lhsh{b}{i}") for i in range(2)]
            for i in range(2):
                nc.vector.tensor_tensor(lhsh[i][:], lhs[i][:], bcr[i][:],
                                        op=OP.mult)
            return lhsh

        def proj_mm(b, lhsh):
            for r in range(4):
                psp = ps_mi.tile([128, 256], F32, tag="mi", name=f"pp{b}{r}")
                for i in range(2):
                    nc.tensor.matmul(psp[:],
                                     lhsh[i][:, r * 128:(r + 1) * 128],
                                     pwt_sb[i][:],
                                     start=(i == 0), stop=(i == 1))
                ot = pool.tile([128, 256], F32, tag="ot", bufs=2,
                               name=f"ot{b}{r}")
                nc.vector.tensor_tensor(ot[:], psp[:], pb_sb[:], op=OP.add)
                nc.sync.dma_start(out_d[b, r * 128:(r + 1) * 128, :], ot[:])

        # pipeline: prep(0) standalone; prep(b+1) + proj(b-1) interleave
        # into attention(b)'s qc loop so PE/ACT/DVE never drain. proj's
        # latency chain (DMAs/reciprocal/broadcast) runs early (qc=1);
        # its matmuls run late (qc=7) when the chain has resolved.
        ctx0, pchunks0 = prep_chunks(0)
        for h in pchunks0:
            h()
        ctxs = {0: ctx0}
        lhsh_pending = {}
        for b in range(B):
            hooks = {}
            if b + 1 < B:
                load_xt(b + 1)
                ctxs[b + 1], pc = prep_chunks(b + 1)
                for qc, fn in zip((3, 4, 5, 6), pc):
                    hooks.setdefault(qc, []).append(fn)
            if b >= 1:
                def _pre(bb=b - 1):
                    lhsh_pending[bb] = proj_pre(bb)
                def _mm(bb=b - 1):
                    proj_mm(bb, lhsh_pending.pop(bb))
                hooks.setdefault(4, []).append(_pre)
                hooks.setdefault(7, []).append(_mm)
            attention_batch(b, ctxs.pop(b), hooks)
            exchange_batch(b)
        proj_mm(B - 1, proj_pre(B - 1))

    nc.compile()
    return nc


def _host_prep(x, relative_pos, q_w, k_w, v_w, proj_w, proj_b, sr_w, sr_b,
               bn_gamma, bn_beta, bn_mean, bn_var):
    f = np.float32
    h = np.float16
    x = np.asarray(x, f)
    relative_pos = np.asarray(relative_pos, f)
    scale = np.float32(DH ** -0.5)

    xt = np.ascontiguousarray(x.transpose(0, 2, 1)).astype(h).reshape(B, 2, 128, N)

    a = (np.asarray(bn_gamma, f) / np.sqrt(np.asarray(bn_var, f) + BN_EPS)).astype(f)
    b_eff = ((np.asarray(sr_b, f) - np.asarray(bn_mean, f)) * a
             + np.asarray(bn_beta, f)).astype(f)
    w_eff = (np.asarray(sr_w, f)[:, 0, :, :].reshape(C, 4) * a[:, None]).astype(f)
    tapw = w_eff.reshape(2, 128, 4)

    v_w = np.asarray(v_w, f)
    proj_w = np.asarray(proj_w, f)
    vb_full = v_w @ b_eff
    pb_eff = (np.asarray(proj_b, f) + proj_w @ vb_full).astype(f)

    pwt = np.ascontiguousarray(proj_w.T).astype(h).reshape(2, 128, 256)
    pbrep = np.tile(pb_eff.reshape(1, 256), (128, 1)).astype(f)

    in_maps = []
    for hh in range(N_CORES):
        qwT_rep = np.tile(
            np.ascontiguousarray((np.asarray(q_w, f)[hh * 32:(hh + 1) * 32, :]
                                  * scale).T), (1, 4)).astype(h).reshape(2, 128, 128)
        kwT = np.ascontiguousarray(
            np.asarray(k_w, f)[hh * 32:(hh + 1) * 32, :].T).astype(h).reshape(2, 128, 32)
        vwT = np.ascontiguousarray(
            v_w[hh * 32:(hh + 1) * 32, :].T).astype(h).reshape(2, 128, 32)
        er = np.exp(relative_pos[hh]).T.astype(bfloat16)   # [NK, N]
        # [g][p][qc*1024 + u*512 + q] = er[(2g+u)*128+p][qc*512+q]
        erp = np.ascontiguousarray(
            er.reshape(4, 2, 128, 8, 512).transpose(0, 2, 3, 1, 4)
            .reshape(4, 128, 2 * N))
        in_maps.append({
            "xt": xt, "erp": erp, "qw": np.ascontiguousarray(qwT_rep),
            "kw": kwT, "vw": vwT, "tapw": np.ascontiguousarray(tapw),
            "pwt": np.ascontiguousarray(pwt), "pbrep": pbrep,
        })
    return in_maps


def run_once(inputs, trace=False, trace_kwargs=None):
    if trace:
        try:
            import antenv.axon_hooks  # noqa: F401
        except ImportError:
            trace = False
    if "nc" not in _CACHE:
        _CACHE["nc"] = _build_nc()
    nc = _CACHE["nc"]
    in_maps = _host_prep(
        inputs["x"], inputs["relative_pos"], inputs["q_w"], inputs["k_w"],
        inputs["v_w"], inputs["proj_w"], inputs["proj_b"], inputs["sr_w"],
        inputs["sr_b"], inputs["bn_gamma"], inputs["bn_beta"],
        inputs["bn_mean"], inputs["bn_var"])
    res = run_bass_kernel_spmd(nc, in_maps, core_ids=list(range(N_CORES)),
                               trace=trace, **(trace_kwargs or {}))
    out = np.zeros((B, N, C), np.float32)
    for d in range(N_CORES):
        r = res.results[d]["out"]          # [B, 512, 256]
        for b in range(B):
            out[b, d * 512:(d + 1) * 512, :] = r[b]
    return out, res


def kernel(**inputs) -> np.ndarray:
    out, _ = run_once(inputs, trace=False)
    return out
